# revision 17
# baseline (speedup 1.0000x reference)
"""Trainium2 Bass kernel for nn_Decoder2 (sequential LSTM music decoder).

Strategy (per sharding hint): single-core persistent kernel — all weights
resident in SBUF, state kept as column vectors across partitions, fully
unrolled 50-step chord chain + 400-step melody chain with host-side weight
folding. Inputs are replicated across the 8 cores (SPMD, identical program);
core 0's output is returned.

Host-side folds (weights only; z-dependent vectors are computed on device):
  - chord:  ch_emb(t) = F @ chord_pred(t) + kz_c,  F = cd_wA @ ce_w,
            kz_c = cd_wB @ z + cd_wA @ ce_b + cd_b  (on device, once).
  - melody: x(s+1) = P @ mel_pred(s) + cvec(t),  P = md_wA @ me_w,
            cvec(t) = md_wB @ ch_emb(t) + kz_m,
            kz_m = md_wC @ z + md_wA @ me_b + md_b  (on device, once).
  - biases folded into row-100 of the stationaries; state rhs vectors carry a
    constant 1.0 in row 100. Compute engines need 32-aligned partition bases,
    so row 100 is produced arithmetically: bootstrap [z;1] / e100 columns via
    K=1 matmuls from a [1,M] row, and keep the 1 alive through the kz/cvec
    chain (kz vectors have a 1 in row 100 via an e100 column in their
    stationaries; intermediate psums have a zero column there).
  - matmuls (LDWEIGHTS) have only 2 HW sync-wait slots, so every loop matmul
    must depend on at most {one DMA-written weight tile, DVE-written data}.
  - g-gate blocks pre-scaled by 2 so one sigmoid serves all 4 gates
    (tanh(x) = 2*sigmoid(2x) - 1, exact identity).
"""
import numpy as np

H, H2, CP, MP, NK, NM = 100, 50, 25, 130, 24, 7
N_OUTER, N_INNER = 50, 8
N_CORES = 8

FP = np.float32


# ---------------------------------------------------------------- packing ---

def pack_inputs(inp):
    """Fold/transpose raw weights into the DRAM layout the kernel expects."""
    g = {k: np.asarray(v, np.float64) for k, v in inp.items()}

    def scale_g(w400):
        w = w400.copy()
        w[200:300] *= 2.0
        return w

    b_c = scale_g(g['cl_bih'] + g['cl_bhh'])
    b_m = scale_g(g['ml_bih'] + g['ml_bhh'])
    cl_wih, cl_whh = scale_g(g['cl_wih']), scale_g(g['cl_whh'])
    ml_wih, ml_whh = scale_g(g['ml_wih']), scale_g(g['ml_whh'])

    cd_wA, cd_wB = g['cd_w'][:, :H], g['cd_w'][:, H:]
    F = cd_wA @ g['ce_w']
    kc_bias = cd_wA @ g['ce_b'] + g['cd_b']

    md_wA, md_wB, md_wC = g['md_w'][:, :H], g['md_w'][:, H:2 * H], g['md_w'][:, 2 * H:]
    P = md_wA @ g['me_w']                      # [100,130]
    km_bias = md_wA @ g['me_b'] + g['md_b']

    def zcol(m, rows):
        """append a zero column (keeps row 100 of the psum at 0)."""
        return np.hstack([m, np.zeros((rows, 1))])

    def ecol(m, rows, one_at):
        c = np.zeros((rows, 1))
        c[one_at, 0] = 1.0
        return np.hstack([m, c])

    # kz stationaries get an e100 column: p[100] = ze[100] = 1
    kzc = ecol(np.vstack([cd_wB.T, kc_bias[None]]), 101, 100)       # [101,101]
    kzm = ecol(np.vstack([md_wC.T, km_bias[None]]), 101, 100)       # [101,101]

    p = {
        'w_cwx_c': np.vstack([cl_wih.T, b_c[None]]),        # [101,400]
        'w_cwh_c': cl_whh.T,                                # [100,400]
        'w_cp1': np.vstack([g['cp_w1'].T, g['cp_b1'][None]]),  # [101,100]
        'w_cp2': np.vstack([g['cp_w2'].T, g['cp_b2'][None]]),  # [101,25]
        'w_fce': zcol(F.T, 25),                             # [25,101]
        'w_kzc': kzc, 'w_kzm': kzm,
        'w_mwx': np.vstack([ml_wih.T, b_m[None]]),          # [101,400]
        'w_mwh': ml_whh.T,                                  # [100,400]
        'w_mp1': np.vstack([g['mp_w1'].T, g['mp_b1'][None]]),  # [101,100]
        'w_mp2': np.vstack([g['mp_w2'].T, g['mp_b2'][None]]),  # [101,130]
        'w_pmm1': zcol(P.T[:65], 65),                       # [65,101]
        'w_pmm2': zcol(P.T[65:], 65),                       # [65,101]
        'w_mdb': zcol(md_wB.T, 100),                        # [100,101]
        'w_hd2k': g['key_w2'].T,                            # [50,24]
        'w_hd2m': g['mode_w2'].T,                           # [50,7]
        'w_hd2t': np.hstack([g['tp_w2'].T, g['vl_w2'].T, g['en_w2'].T]),  # [50,3]
        'w_hdbk': g['key_b2'][:, None],                     # [24,1]
        'w_hdbm': g['mode_b2'][:, None],                    # [7,1]
        'w_hdbt': np.concatenate([g['tp_b2'], g['vl_b2'], g['en_b2']])[None],  # [1,3]
        'z': g['z'],
    }
    hd1 = np.zeros((101, 250))
    for i, hd in enumerate(['key', 'mode', 'tp', 'vl', 'en']):
        hd1[:, 50 * i:50 * i + 50] = np.vstack(
            [g[f'{hd}_w1'].T, g[f'{hd}_b1'][None]])
    p['w_hd1'] = hd1
    return {k: np.ascontiguousarray(v, FP) for k, v in p.items()}


# ---------------------------------------------------------------- builder ---

def build(n_outer=N_OUTER, n_inner=N_INNER, n_cores=N_CORES):
    """Build the Bass program. Returns nc."""
    from contextlib import ExitStack
    import concourse.bacc as bacc
    import concourse.mybir as mybir
    import concourse.tile as tile

    fp32 = mybir.dt.float32
    AF = mybir.ActivationFunctionType
    OP = mybir.AluOpType
    n_mel = n_outer * n_inner

    nc = bacc.Bacc("TRN2", target_bir_lowering=False, debug=False,
                   num_devices=n_cores)

    shapes = {
        'w_cwx_c': (101, 400), 'w_cwh_c': (100, 400), 'w_cp1': (101, 100),
        'w_cp2': (101, 25), 'w_fce': (25, 101), 'w_kzc': (101, 101),
        'w_kzm': (101, 101), 'w_mwx': (101, 400), 'w_mwh': (100, 400),
        'w_mp1': (101, 100), 'w_mp2': (101, 130), 'w_pmm1': (65, 101),
        'w_pmm2': (65, 101), 'w_mdb': (100, 101), 'w_hd1': (101, 250),
        'w_hd2k': (50, 24), 'w_hd2m': (50, 7), 'w_hd2t': (50, 3),
        'w_hdbk': (24, 1), 'w_hdbm': (7, 1), 'w_hdbt': (1, 3),
        'z': (1, 100),
    }
    din = {k: nc.dram_tensor(k, s, fp32, kind="ExternalInput").ap()
           for k, s in shapes.items()}
    d_chord = nc.dram_tensor("chord_out", (n_outer, CP), fp32,
                             kind="ExternalOutput").ap()
    d_mel = nc.dram_tensor("mel_out", (n_mel, MP), fp32,
                           kind="ExternalOutput").ap()
    d_key = nc.dram_tensor("key_out", (1, NK), fp32, kind="ExternalOutput").ap()
    d_mode = nc.dram_tensor("mode_out", (1, NM), fp32, kind="ExternalOutput").ap()
    d_bpm = nc.dram_tensor("bpm", (1, 1), fp32, kind="ExternalOutput").ap()
    d_en = nc.dram_tensor("energy", (1, 1), fp32, kind="ExternalOutput").ap()
    d_vl = nc.dram_tensor("valence", (1, 1), fp32, kind="ExternalOutput").ap()

    with ExitStack() as ctx:
        tc = ctx.enter_context(tile.TileContext(nc))
        wp = ctx.enter_context(tc.tile_pool(name="w", bufs=1))
        sp = ctx.enter_context(tc.tile_pool(name="state", bufs=1))
        pp = ctx.enter_context(tc.tile_pool(name="ps", bufs=1, space="PSUM"))

        # --- weight tiles (DMA once) ---
        # Tiles read by steady-state matmuls are staged through a one-time
        # DVE copy: a matmul's waits then merge into the DVE sem instead of
        # keeping a forwarded DMA-queue wait alive forever (LDWEIGHTS has
        # only 2 HW sync-wait slots).
        wt = {}
        for k in shapes:
            if k == 'z':
                continue
            K, M = shapes[k]
            stg = wp.tile([K, M], fp32, name=f"stg_{k}")
            nc.sync.dma_start(out=stg[0:K, 0:M], in_=din[k])
            t = wp.tile([K, M], fp32, name=f"t_{k}")
            nc.vector.tensor_copy(t[0:K, 0:M], stg[0:K, 0:M])
            wt[k] = t

        # --- state tiles ---
        ze = sp.tile([101, 1], fp32, name="ze")
        che = sp.tile([101, 1], fp32, name="che")
        hce = sp.tile([101, 1], fp32, name="hce")
        hme = sp.tile([101, 1], fp32, name="hme")
        xme = sp.tile([101, 1], fp32, name="xme")
        rce = sp.tile([101, 1], fp32, name="rce")
        rme = sp.tile([101, 1], fp32, name="rme")
        c_c = sp.tile([100, 1], fp32, name="c_c")
        c_m = sp.tile([100, 1], fp32, name="c_m")
        kz_c = sp.tile([101, 1], fp32, name="kz_c")
        kz_m = sp.tile([101, 1], fp32, name="kz_m")
        cvec = [sp.tile([101, 1], fp32, name=f"cvec{b}") for b in range(2)]
        e100 = sp.tile([101, 1], fp32, name="e100")
        seed = sp.tile([1, 1], fp32, name="seed")
        zrow = sp.tile([1, 101], fp32, name="zrow")
        erow = sp.tile([1, 101], fp32, name="erow")
        chord_sb = sp.tile([25, n_outer], fp32, name="chord_sb")
        mel_sb = sp.tile([65, 2 * n_mel], fp32, name="mel_sb")
        rh = sp.tile([50, 5], fp32, name="rh")
        key_sb = sp.tile([24, 1], fp32, name="key_sb")
        mode_sb = sp.tile([7, 1], fp32, name="mode_sb")
        bpm_sb = sp.tile([1, 3], fp32, name="bpm_sb")
        # persistent LSTM elementwise temps (one set per chain)
        sg_c = sp.tile([100, 4], fp32, name="sg_c")
        sg_m = sp.tile([100, 4], fp32, name="sg_m")
        gin_c = sp.tile([100, 4], fp32, name="gin_c")
        gin_m = sp.tile([100, 4], fp32, name="gin_m")
        tmp_c = [sp.tile([100, 1], fp32, name=f"tmp_c{i}") for i in range(3)]
        tmp_m = [sp.tile([100, 1], fp32, name=f"tmp_m{i}") for i in range(3)]

        # persistent PSUM tiles (8 banks): allocating per-step from a pool
        # would attach slot-release waits ({PE writers, ACT readers}) to the
        # first matmul of every step, busting the 2-slot sync-wait limit.
        gm = [pp.tile([100, 4], fp32, tag=f"gm{i}", name=f"gm{i}")
              for i in range(2)]
        pgc = pp.tile([100, 5], fp32, tag="pgc", name="pgc")
        p_c1 = pp.tile([101, 1], fp32, tag="pc1", name="p_c1")
        p_m1 = pp.tile([101, 1], fp32, tag="pm1", name="p_m1")
        p_m2 = pp.tile([65, 2], fp32, tag="pm2", name="p_m2")
        p_px = p_m1
        p_s = pp.tile([25, 3], fp32, tag="psm", name="p_s")
        p_hd = pp.tile([50, 5], fp32, tag="phd", name="p_hd")

        # --- bootstrap [z;1] and e100 columns (steady-state compute must not
        # read DMA-written data, so build them via K=1 matmuls) ---
        z_stg = sp.tile([1, 100], fp32, name="z_stg")
        nc.sync.dma_start(out=z_stg, in_=din['z'])
        nc.vector.tensor_copy(zrow[0:1, 0:100], z_stg)
        nc.vector.memset(zrow[0:1, 100:101], 1.0)
        nc.vector.memset(seed, 1.0)
        nc.vector.memset(erow[0:1, 0:100], 0.0)
        nc.vector.memset(erow[0:1, 100:101], 1.0)
        nc.tensor.matmul(p_c1, zrow, seed)
        nc.vector.tensor_copy(ze, p_c1)
        nc.tensor.matmul(p_c1, erow, seed)
        nc.vector.tensor_copy(e100, p_c1)
        # h/r state init: zeros with a 1.0 in row 100 (rows 0-99 rewritten per
        # step, row 100 persists)
        for t_ in (hce, hme, rce, rme):
            nc.vector.tensor_copy(t_, e100)
        nc.vector.memset(c_c, 0.0)
        nc.vector.memset(c_m, 0.0)

        # --- init columns (z-dependent; stationaries carry an e100 column so
        # row 100 of kz comes out as 1.0) ---
        nc.tensor.matmul(p_c1, wt['w_kzc'][0:101, 0:101], ze[0:101, 0:1])
        nc.vector.tensor_copy(kz_c, p_c1)
        nc.tensor.matmul(p_c1, wt['w_kzm'][0:101, 0:101], ze[0:101, 0:1])
        nc.vector.tensor_copy(kz_m, p_c1)

        # --- heads ---
        for i in range(5):
            nc.tensor.matmul(p_hd[0:50, i:i + 1],
                             wt['w_hd1'][0:101, 50 * i:50 * i + 50],
                             ze[0:101, 0:1])
        nc.vector.tensor_scalar(rh[0:50, 0:5], p_hd[0:50, 0:5], 0.0, None,
                                OP.max)
        nc.tensor.matmul(p_s[0:24, 0:1], wt['w_hd2k'][0:50, 0:24],
                         rh[0:50, 0:1])
        nc.vector.tensor_add(key_sb, p_s[0:24, 0:1], wt['w_hdbk'][0:24, 0:1])
        nc.tensor.matmul(p_s[0:7, 1:2], wt['w_hd2m'][0:50, 0:7], rh[0:50, 1:2])
        nc.vector.tensor_add(mode_sb, p_s[0:7, 1:2], wt['w_hdbm'][0:7, 0:1])
        for j in range(3):
            nc.tensor.matmul(p_s[0:1, j:j + 1], wt['w_hd2t'][0:50, j:j + 1],
                             rh[0:50, 2 + j:3 + j])
        # bpm_sb cols: 0=bpm, 1=valence, 2=energy (all clipped)
        nc.vector.tensor_add(bpm_sb, p_s[0:1, 0:3], wt['w_hdbt'][0:1, 0:3])
        nc.vector.tensor_scalar(bpm_sb, bpm_sb, 0.0, 1.0, OP.max, OP.min)
        nc.vector.tensor_scalar(bpm_sb[0:1, 0:1], bpm_sb[0:1, 0:1],
                                30.0, 70.0, OP.mult, OP.add)
        nc.sync.dma_start(out=d_key.rearrange("o k -> k o"), in_=key_sb)
        nc.sync.dma_start(out=d_mode.rearrange("o k -> k o"), in_=mode_sb)
        nc.sync.dma_start(out=d_bpm, in_=bpm_sb[0:1, 0:1])
        nc.sync.dma_start(out=d_vl, in_=bpm_sb[0:1, 1:2])
        nc.sync.dma_start(out=d_en, in_=bpm_sb[0:1, 2:3])

        def lstm_elem(pg, c, h_dst, sg, gin, tmps):
            """gates psum [100,4] cols (i,f,g*2,o) -> update c, h_dst[0:100].
            The psum is bounced through SBUF on DVE so matmuls never inherit
            an ACT wait (LDWEIGHTS has a single HW sync-wait slot)."""
            tg, fc, tc_ = tmps
            nc.vector.tensor_copy(gin, pg)
            nc.scalar.activation(sg, gin, AF.Sigmoid)
            nc.vector.tensor_scalar(tg, sg[:, 2:3], 2.0, -1.0, OP.mult, OP.add)
            nc.vector.tensor_scalar(fc, c, sg[:, 1:2], None, OP.mult)
            nc.vector.scalar_tensor_tensor(c, tg, sg[:, 0:1], fc,
                                           OP.mult, OP.add)
            nc.scalar.activation(tc_, c, AF.Tanh)
            nc.vector.tensor_scalar(h_dst[0:100, 0:1], tc_, sg[:, 3:4],
                                    None, OP.mult)

        for t in range(n_outer):
            # ---- chord step t ----
            rhs_x = ze if t == 0 else che
            pg = pgc
            for j in range(4):
                nc.tensor.matmul(pg[:, j:j + 1],
                                 wt['w_cwx_c'][0:101, 100 * j:100 * j + 100],
                                 rhs_x[0:101, 0:1], start=True, stop=(t == 0))
                if t > 0:
                    nc.tensor.matmul(pg[:, j:j + 1],
                                     wt['w_cwh_c'][0:100, 100 * j:100 * j + 100],
                                     hce[0:100, 0:1], start=False, stop=True)
            lstm_elem(pg[0:100, 0:4], c_c, hce, sg_c, gin_c, tmp_c)
            nc.tensor.matmul(p_m1[0:100, 0:1], wt['w_cp1'][0:101, 0:100],
                             hce[0:101, 0:1])
            nc.vector.tensor_scalar(rce[0:100, 0:1], p_m1[0:100, 0:1], 0.0,
                                    None, OP.max)
            nc.tensor.matmul(p_s[0:25, 0:1], wt['w_cp2'][0:101, 0:25],
                             rce[0:101, 0:1])
            nc.vector.tensor_copy(chord_sb[0:25, t:t + 1], p_s[0:25, 0:1])
            nc.tensor.matmul(p_c1, wt['w_fce'][0:25, 0:101],
                             chord_sb[0:25, t:t + 1])
            nc.vector.scalar_tensor_tensor(che, p_c1, 1.0, kz_c,
                                           OP.mult, OP.add)

            # ---- cvec for outer t (row 100 = 0 + kz_m[100] = 1) ----
            cv = cvec[t % 2]
            nc.tensor.matmul(p_c1, wt['w_mdb'][0:100, 0:101], che[0:100, 0:1])
            nc.vector.scalar_tensor_tensor(cv, p_c1, 1.0, kz_m, OP.mult, OP.add)

            # ---- melody steps ----
            for s in range(n_inner * t, n_inner * (t + 1)):
                pgm = gm[s % 2]
                rhs_m = che if s == 0 else xme
                for j in range(4):
                    nc.tensor.matmul(
                        pgm[:, j:j + 1],
                        wt['w_mwx'][0:101, 100 * j:100 * j + 100],
                        rhs_m[0:101, 0:1], start=True, stop=(s == 0))
                    if s > 0:
                        nc.tensor.matmul(
                            pgm[:, j:j + 1],
                            wt['w_mwh'][0:100, 100 * j:100 * j + 100],
                            hme[0:100, 0:1], start=False, stop=True)
                lstm_elem(pgm, c_m, hme, sg_m, gin_m, tmp_m)
                nc.tensor.matmul(p_m1[0:100, 0:1], wt['w_mp1'][0:101, 0:100],
                                 hme[0:101, 0:1])
                nc.vector.tensor_scalar(rme[0:100, 0:1], p_m1[0:100, 0:1],
                                        0.0, None, OP.max)
                nc.tensor.matmul(p_m2[:, 0:1], wt['w_mp2'][0:101, 0:65],
                                 rme[0:101, 0:1])
                nc.tensor.matmul(p_m2[:, 1:2], wt['w_mp2'][0:101, 65:130],
                                 rme[0:101, 0:1])
                nc.vector.tensor_copy(mel_sb[0:65, 2 * s:2 * s + 2], p_m2)
                # next x = P @ mel_pred + cvec  (row 100 = 0 + cv[100] = 1)
                nc.tensor.matmul(p_px, wt['w_pmm1'][0:65, 0:101],
                                 mel_sb[0:65, 2 * s:2 * s + 1],
                                 start=True, stop=False)
                nc.tensor.matmul(p_px, wt['w_pmm2'][0:65, 0:101],
                                 mel_sb[0:65, 2 * s + 1:2 * s + 2],
                                 start=False, stop=True)
                nc.vector.scalar_tensor_tensor(xme, p_px, 1.0, cv,
                                               OP.mult, OP.add)

        # ---- outputs ----
        nc.sync.dma_start(out=d_chord.rearrange("t p -> p t"),
                          in_=chord_sb[0:25, 0:n_outer])
        nc.sync.dma_start(out=d_mel.rearrange("s (two p) -> p s two", two=2),
                          in_=mel_sb[0:65, 0:2 * n_mel].rearrange(
                              "p (s two) -> p s two", two=2))
    nc.compile()
    return nc


# ----------------------------------------------------------------- runner ---

_CACHE = {}


def _enable_ntff_tracing():
    """Register the axon NTFF profile hook if the container's antenv lacks it,
    and neuter the bucket upload. Only needed for trace=True runs."""
    import sys
    import types
    try:
        from antenv.axon_hooks import get_axon_ntff_profile_hook  # noqa: F401
        have = True
    except ImportError:
        have = False
    if not have:
        from trn_agent_boot.trn_boot import _ntff_profile_via_ctypes
        hook = _ntff_profile_via_ctypes('/opt/axon/libaxon_pjrt.so')
        mod = types.ModuleType('antenv.axon_hooks')
        mod.get_axon_ntff_profile_hook = lambda: hook
        mod.set_axon_ntff_profile_hook = lambda h: None
        sys.modules['antenv.axon_hooks'] = mod
    import concourse.bass_utils as bu
    bu.upload_artifacts = lambda tmpdir: f"local://{tmpdir}"


def _get_nc(n_outer, n_inner, n_cores):
    k = (n_outer, n_inner, n_cores)
    if k not in _CACHE:
        _CACHE[k] = build(n_outer, n_inner, n_cores)
    return _CACHE[k]


def run(inputs, n_outer=N_OUTER, n_inner=N_INNER, n_cores=N_CORES,
        trace=False):
    """Run on hardware; returns (outputs_tuple, BassKernelResults)."""
    from concourse.bass_utils import run_bass_kernel_spmd
    if trace:
        try:
            _enable_ntff_tracing()
        except Exception as e:  # degrade to no-trace
            print(f'ntff tracing unavailable: {e}')
            trace = False
    nc = _get_nc(n_outer, n_inner, n_cores)
    in_map = pack_inputs(inputs)
    res = run_bass_kernel_spmd(nc, [dict(in_map) for _ in range(n_cores)],
                               core_ids=list(range(n_cores)), trace=trace)
    r = res.results[0]
    n_mel = n_outer * n_inner
    out = (r['chord_out'].reshape(n_outer, 1, CP).astype(FP),
           r['mel_out'].reshape(n_mel, 1, MP).astype(FP),
           r['key_out'].astype(FP), r['mode_out'].astype(FP),
           r['bpm'].astype(FP), r['energy'].astype(FP),
           r['valence'].astype(FP))
    return out, res


def kernel(**inputs):
    out, _ = run(inputs)
    return out


# revision 20
# speedup vs baseline: 1.7855x; 1.7855x over previous
"""Trainium2 Bass kernel for nn_Decoder2 (sequential LSTM music decoder).

Strategy (per sharding hint): single-core persistent kernel — all weights
resident in SBUF, state kept as column vectors across partitions, fully
unrolled 50-step chord chain + 400-step melody chain with host-side weight
folding. Inputs are replicated across the 8 cores (SPMD, identical program);
core 0's output is returned.

Host-side folds (weights only; z-dependent vectors are computed on device):
  - chord:  ch_emb(t) = F @ chord_pred(t) + kz_c,  F = cd_wA @ ce_w,
            kz_c = cd_wB @ z + cd_wA @ ce_b + cd_b  (on device, once).
  - melody: x(s+1) = P @ mel_pred(s) + cvec(t),  P = md_wA @ me_w,
            cvec(t) = md_wB @ ch_emb(t) + kz_m,
            kz_m = md_wC @ z + md_wA @ me_b + md_b  (on device, once).
  - biases folded into row-100 of the stationaries; state rhs vectors carry a
    constant 1.0 in row 100. Compute engines need 32-aligned partition bases,
    so row 100 is produced arithmetically: bootstrap [z;1] / e100 columns via
    K=1 matmuls from a [1,M] row, and keep the 1 alive through the kz/cvec
    chain (kz vectors have a 1 in row 100 via an e100 column in their
    stationaries; intermediate psums have a zero column there).
  - matmuls (LDWEIGHTS) have only 2 HW sync-wait slots, so every loop matmul
    must depend on at most {one DMA-written weight tile, DVE-written data}.
  - g-gate blocks pre-scaled by 2 so one sigmoid serves all 4 gates
    (tanh(x) = 2*sigmoid(2x) - 1, exact identity).
"""
import numpy as np

H, H2, CP, MP, NK, NM = 100, 50, 25, 130, 24, 7
N_OUTER, N_INNER = 50, 8
N_CORES = 8

FP = np.float32


# ---------------------------------------------------------------- packing ---

LOOP_W = {'w_cwx_c', 'w_cwh_c', 'w_cp1', 'w_cp2', 'w_fce', 'w_mwx', 'w_mwh',
          'w_mp1', 'w_mp2', 'w_pmm1', 'w_pmm2', 'w_mdb'}


def pack_inputs(inp, mm_dtype=np.float16):
    """Fold/transpose raw weights into the DRAM layout the kernel expects.
    Loop-matmul weights are cast to mm_dtype (fp32 matmuls run at 1/4 rate on
    the PE); init/head weights stay fp32."""
    g = {k: np.asarray(v, np.float64) for k, v in inp.items()}

    def scale_g(w400):
        w = w400.copy()
        w[200:300] *= 2.0
        return w

    b_c = scale_g(g['cl_bih'] + g['cl_bhh'])
    b_m = scale_g(g['ml_bih'] + g['ml_bhh'])
    cl_wih, cl_whh = scale_g(g['cl_wih']), scale_g(g['cl_whh'])
    ml_wih, ml_whh = scale_g(g['ml_wih']), scale_g(g['ml_whh'])

    cd_wA, cd_wB = g['cd_w'][:, :H], g['cd_w'][:, H:]
    F = cd_wA @ g['ce_w']
    kc_bias = cd_wA @ g['ce_b'] + g['cd_b']

    md_wA, md_wB, md_wC = g['md_w'][:, :H], g['md_w'][:, H:2 * H], g['md_w'][:, 2 * H:]
    P = md_wA @ g['me_w']                      # [100,130]
    km_bias = md_wA @ g['me_b'] + g['md_b']

    def zcol(m, rows):
        """append a zero column (keeps row 100 of the psum at 0)."""
        return np.hstack([m, np.zeros((rows, 1))])

    def ecol(m, rows, one_at):
        c = np.zeros((rows, 1))
        c[one_at, 0] = 1.0
        return np.hstack([m, c])

    # kz stationaries get an e100 column: p[100] = ze[100] = 1
    kzc = ecol(np.vstack([cd_wB.T, kc_bias[None]]), 101, 100)       # [101,101]
    kzm = ecol(np.vstack([md_wC.T, km_bias[None]]), 101, 100)       # [101,101]

    p = {
        'w_cwx_c': np.vstack([cl_wih.T, b_c[None]]),        # [101,400]
        'w_cwh_c': cl_whh.T,                                # [100,400]
        'w_cp1': np.vstack([g['cp_w1'].T, g['cp_b1'][None]]),  # [101,100]
        'w_cp2': np.vstack([g['cp_w2'].T, g['cp_b2'][None]]),  # [101,25]
        'w_fce': zcol(F.T, 25),                             # [25,101]
        'w_kzc': kzc, 'w_kzm': kzm,
        'w_mwx': np.vstack([ml_wih.T, b_m[None]]),          # [101,400]
        'w_mwh': ml_whh.T,                                  # [100,400]
        'w_mp1': np.vstack([g['mp_w1'].T, g['mp_b1'][None]]),  # [101,100]
        'w_mp2': np.vstack([g['mp_w2'].T, g['mp_b2'][None]]),  # [101,130]
        'w_pmm1': zcol(P.T[:65], 65),                       # [65,101]
        'w_pmm2': zcol(P.T[65:], 65),                       # [65,101]
        'w_mdb': zcol(md_wB.T, 100),                        # [100,101]
        'w_hd2k': g['key_w2'].T,                            # [50,24]
        'w_hd2m': g['mode_w2'].T,                           # [50,7]
        'w_hd2t': np.hstack([g['tp_w2'].T, g['vl_w2'].T, g['en_w2'].T]),  # [50,3]
        'w_hdbk': g['key_b2'][:, None],                     # [24,1]
        'w_hdbm': g['mode_b2'][:, None],                    # [7,1]
        'w_hdbt': np.concatenate([g['tp_b2'], g['vl_b2'], g['en_b2']])[None],  # [1,3]
        'z': g['z'],
    }
    hd1 = np.zeros((101, 250))
    for i, hd in enumerate(['key', 'mode', 'tp', 'vl', 'en']):
        hd1[:, 50 * i:50 * i + 50] = np.vstack(
            [g[f'{hd}_w1'].T, g[f'{hd}_b1'][None]])
    p['w_hd1'] = hd1
    return {k: np.ascontiguousarray(v, mm_dtype if k in LOOP_W else FP)
            for k, v in p.items()}


# ---------------------------------------------------------------- builder ---

def build(n_outer=N_OUTER, n_inner=N_INNER, n_cores=N_CORES,
          mm_dtype='float16'):
    """Build the Bass program. Returns nc."""
    from contextlib import ExitStack
    import concourse.bacc as bacc
    import concourse.mybir as mybir
    import concourse.tile as tile

    fp32 = mybir.dt.float32
    fpm = getattr(mybir.dt, mm_dtype)
    AF = mybir.ActivationFunctionType
    OP = mybir.AluOpType
    n_mel = n_outer * n_inner

    nc = bacc.Bacc("TRN2", target_bir_lowering=False, debug=False,
                   num_devices=n_cores)

    shapes = {
        'w_cwx_c': (101, 400), 'w_cwh_c': (100, 400), 'w_cp1': (101, 100),
        'w_cp2': (101, 25), 'w_fce': (25, 101), 'w_kzc': (101, 101),
        'w_kzm': (101, 101), 'w_mwx': (101, 400), 'w_mwh': (100, 400),
        'w_mp1': (101, 100), 'w_mp2': (101, 130), 'w_pmm1': (65, 101),
        'w_pmm2': (65, 101), 'w_mdb': (100, 101), 'w_hd1': (101, 250),
        'w_hd2k': (50, 24), 'w_hd2m': (50, 7), 'w_hd2t': (50, 3),
        'w_hdbk': (24, 1), 'w_hdbm': (7, 1), 'w_hdbt': (1, 3),
        'z': (1, 100),
    }
    din = {k: nc.dram_tensor(k, s, fpm if k in LOOP_W else fp32,
                             kind="ExternalInput").ap()
           for k, s in shapes.items()}
    d_chord = nc.dram_tensor("chord_out", (n_outer, CP), fp32,
                             kind="ExternalOutput").ap()
    d_mel = nc.dram_tensor("mel_out", (n_mel, MP), fp32,
                           kind="ExternalOutput").ap()
    d_key = nc.dram_tensor("key_out", (1, NK), fp32, kind="ExternalOutput").ap()
    d_mode = nc.dram_tensor("mode_out", (1, NM), fp32, kind="ExternalOutput").ap()
    d_bpm = nc.dram_tensor("bpm", (1, 1), fp32, kind="ExternalOutput").ap()
    d_en = nc.dram_tensor("energy", (1, 1), fp32, kind="ExternalOutput").ap()
    d_vl = nc.dram_tensor("valence", (1, 1), fp32, kind="ExternalOutput").ap()

    with ExitStack() as ctx:
        tc = ctx.enter_context(tile.TileContext(nc))
        wp = ctx.enter_context(tc.tile_pool(name="w", bufs=1))
        sp = ctx.enter_context(tc.tile_pool(name="state", bufs=1))
        pp = ctx.enter_context(tc.tile_pool(name="ps", bufs=1, space="PSUM"))

        # --- weight tiles (DMA once) ---
        # Tiles read by steady-state matmuls are staged through a one-time
        # DVE copy: a matmul's waits then merge into the DVE sem instead of
        # keeping a forwarded DMA-queue wait alive forever (LDWEIGHTS has
        # only 2 HW sync-wait slots).
        wt = {}
        for k in shapes:
            if k == 'z':
                continue
            K, M = shapes[k]
            dt_k = fpm if k in LOOP_W else fp32
            stg = wp.tile([K, M], dt_k, name=f"stg_{k}")
            nc.sync.dma_start(out=stg[0:K, 0:M], in_=din[k])
            t = wp.tile([K, M], dt_k, name=f"t_{k}")
            nc.vector.tensor_copy(t[0:K, 0:M], stg[0:K, 0:M])
            wt[k] = t

        # --- state tiles ---
        ze = sp.tile([101, 1], fp32, name="ze")
        ze16 = sp.tile([101, 1], fpm, name="ze16")
        che = sp.tile([101, 1], fpm, name="che")
        hce = sp.tile([101, 1], fpm, name="hce")
        hme = sp.tile([101, 1], fpm, name="hme")
        xme = sp.tile([101, 1], fpm, name="xme")
        rce = sp.tile([101, 1], fpm, name="rce")
        rme = sp.tile([101, 1], fpm, name="rme")
        c_c = sp.tile([100, 1], fp32, name="c_c")
        c_m = sp.tile([100, 1], fp32, name="c_m")
        kz_c = sp.tile([101, 1], fp32, name="kz_c")
        kz_m = sp.tile([101, 1], fp32, name="kz_m")
        cvec = [sp.tile([101, 1], fp32, name=f"cvec{b}") for b in range(2)]
        e100 = sp.tile([101, 1], fp32, name="e100")
        seed = sp.tile([1, 1], fp32, name="seed")
        zrow = sp.tile([1, 101], fp32, name="zrow")
        erow = sp.tile([1, 101], fp32, name="erow")
        chord_sb = sp.tile([25, n_outer], fpm, name="chord_sb")
        chord_o32 = sp.tile([25, n_outer], fp32, name="chord_o32")
        mel_sb = sp.tile([65, 2 * n_mel], fpm, name="mel_sb")
        mel_o32 = sp.tile([65, 2 * n_mel], fp32, name="mel_o32")
        rh = sp.tile([50, 5], fp32, name="rh")
        key_sb = sp.tile([24, 1], fp32, name="key_sb")
        mode_sb = sp.tile([7, 1], fp32, name="mode_sb")
        bpm_sb = sp.tile([1, 3], fp32, name="bpm_sb")
        # persistent LSTM elementwise temps (one set per chain)
        sg_c = sp.tile([100, 4], fp32, name="sg_c")
        sg_m = sp.tile([100, 4], fp32, name="sg_m")
        gin_c = sp.tile([100, 4], fp32, name="gin_c")
        gin_m = sp.tile([100, 4], fp32, name="gin_m")
        tmp_c = [sp.tile([100, 1], fp32, name=f"tmp_c{i}") for i in range(3)]
        tmp_m = [sp.tile([100, 1], fp32, name=f"tmp_m{i}") for i in range(3)]

        # persistent PSUM tiles (8 banks): allocating per-step from a pool
        # would attach slot-release waits ({PE writers, ACT readers}) to the
        # first matmul of every step, busting the 2-slot sync-wait limit.
        gm = [pp.tile([100, 4], fp32, tag=f"gm{i}", name=f"gm{i}")
              for i in range(2)]
        pgc = pp.tile([100, 5], fp32, tag="pgc", name="pgc")
        p_c1 = pp.tile([101, 1], fp32, tag="pc1", name="p_c1")
        p_m1 = pp.tile([101, 1], fp32, tag="pm1", name="p_m1")
        p_m2 = pp.tile([65, 2], fp32, tag="pm2", name="p_m2")
        p_px = p_m1
        p_s = pp.tile([25, 3], fp32, tag="psm", name="p_s")
        p_hd = pp.tile([50, 5], fp32, tag="phd", name="p_hd")

        # --- bootstrap [z;1] and e100 columns (steady-state compute must not
        # read DMA-written data, so build them via K=1 matmuls) ---
        z_stg = sp.tile([1, 100], fp32, name="z_stg")
        nc.sync.dma_start(out=z_stg, in_=din['z'])
        nc.vector.tensor_copy(zrow[0:1, 0:100], z_stg)
        nc.vector.memset(zrow[0:1, 100:101], 1.0)
        nc.vector.memset(seed, 1.0)
        nc.vector.memset(erow[0:1, 0:100], 0.0)
        nc.vector.memset(erow[0:1, 100:101], 1.0)
        nc.tensor.matmul(p_c1, zrow, seed)
        nc.vector.tensor_copy(ze, p_c1)
        nc.tensor.matmul(p_c1, erow, seed)
        nc.vector.tensor_copy(e100, p_c1)
        nc.vector.tensor_copy(ze16, ze)
        # h/r state init: zeros with a 1.0 in row 100 (rows 0-99 rewritten per
        # step, row 100 persists)
        for t_ in (hce, hme, rce, rme):
            nc.vector.tensor_copy(t_, e100)
        nc.vector.memset(c_c, 0.0)
        nc.vector.memset(c_m, 0.0)

        # --- init columns (z-dependent; stationaries carry an e100 column so
        # row 100 of kz comes out as 1.0) ---
        nc.tensor.matmul(p_c1, wt['w_kzc'][0:101, 0:101], ze[0:101, 0:1])
        nc.vector.tensor_copy(kz_c, p_c1)
        nc.tensor.matmul(p_c1, wt['w_kzm'][0:101, 0:101], ze[0:101, 0:1])
        nc.vector.tensor_copy(kz_m, p_c1)

        # --- heads ---
        for i in range(5):
            nc.tensor.matmul(p_hd[0:50, i:i + 1],
                             wt['w_hd1'][0:101, 50 * i:50 * i + 50],
                             ze[0:101, 0:1])
        nc.vector.tensor_scalar(rh[0:50, 0:5], p_hd[0:50, 0:5], 0.0, None,
                                OP.max)
        nc.tensor.matmul(p_s[0:24, 0:1], wt['w_hd2k'][0:50, 0:24],
                         rh[0:50, 0:1])
        nc.vector.tensor_add(key_sb, p_s[0:24, 0:1], wt['w_hdbk'][0:24, 0:1])
        nc.tensor.matmul(p_s[0:7, 1:2], wt['w_hd2m'][0:50, 0:7], rh[0:50, 1:2])
        nc.vector.tensor_add(mode_sb, p_s[0:7, 1:2], wt['w_hdbm'][0:7, 0:1])
        for j in range(3):
            nc.tensor.matmul(p_s[0:1, j:j + 1], wt['w_hd2t'][0:50, j:j + 1],
                             rh[0:50, 2 + j:3 + j])
        # bpm_sb cols: 0=bpm, 1=valence, 2=energy (all clipped)
        nc.vector.tensor_add(bpm_sb, p_s[0:1, 0:3], wt['w_hdbt'][0:1, 0:3])
        nc.vector.tensor_scalar(bpm_sb, bpm_sb, 0.0, 1.0, OP.max, OP.min)
        nc.vector.tensor_scalar(bpm_sb[0:1, 0:1], bpm_sb[0:1, 0:1],
                                30.0, 70.0, OP.mult, OP.add)
        nc.sync.dma_start(out=d_key.rearrange("o k -> k o"), in_=key_sb)
        nc.sync.dma_start(out=d_mode.rearrange("o k -> k o"), in_=mode_sb)
        nc.sync.dma_start(out=d_bpm, in_=bpm_sb[0:1, 0:1])
        nc.sync.dma_start(out=d_vl, in_=bpm_sb[0:1, 1:2])
        nc.sync.dma_start(out=d_en, in_=bpm_sb[0:1, 2:3])

        def lstm_elem(pg, c, h_dst, sg, gin, tmps):
            """gates psum [100,4] cols (i,f,g*2,o) -> update c, h_dst[0:100].
            The psum is bounced through SBUF on DVE so matmuls never inherit
            an ACT wait (LDWEIGHTS has a single HW sync-wait slot)."""
            tg, fc, tc_ = tmps
            nc.vector.tensor_copy(gin, pg)
            nc.scalar.activation(sg, gin, AF.Sigmoid)
            nc.vector.tensor_scalar(tg, sg[:, 2:3], 2.0, -1.0, OP.mult, OP.add)
            nc.vector.tensor_scalar(fc, c, sg[:, 1:2], None, OP.mult)
            nc.vector.scalar_tensor_tensor(c, tg, sg[:, 0:1], fc,
                                           OP.mult, OP.add)
            nc.scalar.activation(tc_, c, AF.Tanh)
            nc.vector.tensor_scalar(h_dst[0:100, 0:1], tc_, sg[:, 3:4],
                                    None, OP.mult)

        for t in range(n_outer):
            # ---- chord step t ----
            rhs_x = ze16 if t == 0 else che
            pg = pgc
            for j in range(4):
                nc.tensor.matmul(pg[:, j:j + 1],
                                 wt['w_cwx_c'][0:101, 100 * j:100 * j + 100],
                                 rhs_x[0:101, 0:1], start=True, stop=(t == 0))
                if t > 0:
                    nc.tensor.matmul(pg[:, j:j + 1],
                                     wt['w_cwh_c'][0:100, 100 * j:100 * j + 100],
                                     hce[0:100, 0:1], start=False, stop=True)
            lstm_elem(pg[0:100, 0:4], c_c, hce, sg_c, gin_c, tmp_c)
            nc.tensor.matmul(p_m1[0:100, 0:1], wt['w_cp1'][0:101, 0:100],
                             hce[0:101, 0:1])
            nc.vector.tensor_scalar(rce[0:100, 0:1], p_m1[0:100, 0:1], 0.0,
                                    None, OP.max)
            nc.tensor.matmul(p_s[0:25, 0:1], wt['w_cp2'][0:101, 0:25],
                             rce[0:101, 0:1])
            nc.vector.tensor_copy(chord_sb[0:25, t:t + 1], p_s[0:25, 0:1])
            nc.vector.tensor_copy(chord_o32[0:25, t:t + 1], p_s[0:25, 0:1])
            nc.tensor.matmul(p_c1, wt['w_fce'][0:25, 0:101],
                             chord_sb[0:25, t:t + 1])
            nc.vector.scalar_tensor_tensor(che, p_c1, 1.0, kz_c,
                                           OP.mult, OP.add)

            # ---- cvec for outer t (row 100 = 0 + kz_m[100] = 1) ----
            cv = cvec[t % 2]
            nc.tensor.matmul(p_c1, wt['w_mdb'][0:100, 0:101], che[0:100, 0:1])
            nc.vector.scalar_tensor_tensor(cv, p_c1, 1.0, kz_m, OP.mult, OP.add)

            # ---- melody steps ----
            for s in range(n_inner * t, n_inner * (t + 1)):
                pgm = gm[s % 2]
                rhs_m = che if s == 0 else xme
                for j in range(4):
                    nc.tensor.matmul(
                        pgm[:, j:j + 1],
                        wt['w_mwx'][0:101, 100 * j:100 * j + 100],
                        rhs_m[0:101, 0:1], start=True, stop=(s == 0))
                    if s > 0:
                        nc.tensor.matmul(
                            pgm[:, j:j + 1],
                            wt['w_mwh'][0:100, 100 * j:100 * j + 100],
                            hme[0:100, 0:1], start=False, stop=True)
                lstm_elem(pgm, c_m, hme, sg_m, gin_m, tmp_m)
                nc.tensor.matmul(p_m1[0:100, 0:1], wt['w_mp1'][0:101, 0:100],
                                 hme[0:101, 0:1])
                nc.vector.tensor_scalar(rme[0:100, 0:1], p_m1[0:100, 0:1],
                                        0.0, None, OP.max)
                nc.tensor.matmul(p_m2[:, 0:1], wt['w_mp2'][0:101, 0:65],
                                 rme[0:101, 0:1])
                nc.tensor.matmul(p_m2[:, 1:2], wt['w_mp2'][0:101, 65:130],
                                 rme[0:101, 0:1])
                nc.vector.tensor_copy(mel_sb[0:65, 2 * s:2 * s + 2], p_m2)
                nc.vector.tensor_copy(mel_o32[0:65, 2 * s:2 * s + 2], p_m2)
                # next x = P @ mel_pred + cvec  (row 100 = 0 + cv[100] = 1)
                nc.tensor.matmul(p_px, wt['w_pmm1'][0:65, 0:101],
                                 mel_sb[0:65, 2 * s:2 * s + 1],
                                 start=True, stop=False)
                nc.tensor.matmul(p_px, wt['w_pmm2'][0:65, 0:101],
                                 mel_sb[0:65, 2 * s + 1:2 * s + 2],
                                 start=False, stop=True)
                nc.vector.scalar_tensor_tensor(xme, p_px, 1.0, cv,
                                               OP.mult, OP.add)

        # ---- outputs ----
        nc.sync.dma_start(out=d_chord.rearrange("t p -> p t"),
                          in_=chord_o32[0:25, 0:n_outer])
        nc.sync.dma_start(out=d_mel.rearrange("s (two p) -> p s two", two=2),
                          in_=mel_o32[0:65, 0:2 * n_mel].rearrange(
                              "p (s two) -> p s two", two=2))
    nc.compile()
    return nc


# ----------------------------------------------------------------- runner ---

_CACHE = {}


def _enable_ntff_tracing():
    """Register the axon NTFF profile hook if the container's antenv lacks it,
    and neuter the bucket upload. Only needed for trace=True runs."""
    import sys
    import types
    try:
        from antenv.axon_hooks import get_axon_ntff_profile_hook  # noqa: F401
        have = True
    except ImportError:
        have = False
    if not have:
        from trn_agent_boot.trn_boot import _ntff_profile_via_ctypes
        hook = _ntff_profile_via_ctypes('/opt/axon/libaxon_pjrt.so')
        mod = types.ModuleType('antenv.axon_hooks')
        mod.get_axon_ntff_profile_hook = lambda: hook
        mod.set_axon_ntff_profile_hook = lambda h: None
        sys.modules['antenv.axon_hooks'] = mod
    import concourse.bass_utils as bu
    bu.upload_artifacts = lambda tmpdir: f"local://{tmpdir}"


MM_DTYPE = 'float16'


def _get_nc(n_outer, n_inner, n_cores, mm_dtype=MM_DTYPE):
    k = (n_outer, n_inner, n_cores, mm_dtype)
    if k not in _CACHE:
        _CACHE[k] = build(n_outer, n_inner, n_cores, mm_dtype)
    return _CACHE[k]


def run(inputs, n_outer=N_OUTER, n_inner=N_INNER, n_cores=N_CORES,
        trace=False, mm_dtype=MM_DTYPE):
    """Run on hardware; returns (outputs_tuple, BassKernelResults)."""
    import numpy as _np
    from concourse.bass_utils import run_bass_kernel_spmd
    if trace:
        try:
            _enable_ntff_tracing()
        except Exception as e:  # degrade to no-trace
            print(f'ntff tracing unavailable: {e}')
            trace = False
    nc = _get_nc(n_outer, n_inner, n_cores, mm_dtype)
    in_map = pack_inputs(inputs, getattr(_np, mm_dtype))
    res = run_bass_kernel_spmd(nc, [dict(in_map) for _ in range(n_cores)],
                               core_ids=list(range(n_cores)), trace=trace)
    r = res.results[0]
    n_mel = n_outer * n_inner
    out = (r['chord_out'].reshape(n_outer, 1, CP).astype(FP),
           r['mel_out'].reshape(n_mel, 1, MP).astype(FP),
           r['key_out'].astype(FP), r['mode_out'].astype(FP),
           r['bpm'].astype(FP), r['energy'].astype(FP),
           r['valence'].astype(FP))
    return out, res


def kernel(**inputs):
    out, _ = run(inputs)
    return out


# revision 22
# speedup vs baseline: 2.2596x; 1.2655x over previous
"""Trainium2 Bass kernel for nn_Decoder2 (sequential LSTM music decoder).

Strategy (per sharding hint): single-core persistent kernel — all weights
resident in SBUF, state kept as column vectors across partitions, fully
unrolled 50-step chord chain + 400-step melody chain with host-side weight
folding. Inputs are replicated across the 8 cores (SPMD, identical program);
core 0's output is returned.

Host-side folds (weights only; z-dependent vectors are computed on device):
  - chord:  ch_emb(t) = F @ chord_pred(t) + kz_c,  F = cd_wA @ ce_w,
            kz_c = cd_wB @ z + cd_wA @ ce_b + cd_b  (on device, once).
  - melody: x(s+1) = P @ mel_pred(s) + cvec(t),  P = md_wA @ me_w,
            cvec(t) = md_wB @ ch_emb(t) + kz_m,
            kz_m = md_wC @ z + md_wA @ me_b + md_b  (on device, once).
  - biases folded into row-100 of the stationaries; state rhs vectors carry a
    constant 1.0 in row 100. Compute engines need 32-aligned partition bases,
    so row 100 is produced arithmetically: bootstrap [z;1] / e100 columns via
    K=1 matmuls from a [1,M] row, and keep the 1 alive through the kz/cvec
    chain (kz vectors have a 1 in row 100 via an e100 column in their
    stationaries; intermediate psums have a zero column there).
  - matmuls (LDWEIGHTS) have only 2 HW sync-wait slots, so every loop matmul
    must depend on at most {one DMA-written weight tile, DVE-written data}.
  - g-gate blocks pre-scaled by 2 so one sigmoid serves all 4 gates
    (tanh(x) = 2*sigmoid(2x) - 1, exact identity).
"""
import numpy as np

H, H2, CP, MP, NK, NM = 100, 50, 25, 130, 24, 7
N_OUTER, N_INNER = 50, 8
N_CORES = 8

FP = np.float32


# ---------------------------------------------------------------- packing ---

LOOP_W = {'w_cwx_c', 'w_cwh_c', 'w_cp1', 'w_cp2', 'w_fw2', 'w_pw2',
          'w_mwx', 'w_mwh', 'w_mp1', 'w_mp2', 'w_mdb'}


def pack_inputs(inp, mm_dtype=np.float16):
    """Fold/transpose raw weights into the DRAM layout the kernel expects.
    Loop-matmul weights are cast to mm_dtype (fp32 matmuls run at 1/4 rate on
    the PE); init/head weights stay fp32."""
    g = {k: np.asarray(v, np.float64) for k, v in inp.items()}

    def scale_g(w400):
        w = w400.copy()
        w[200:300] *= 2.0
        return w

    b_c = scale_g(g['cl_bih'] + g['cl_bhh'])
    b_m = scale_g(g['ml_bih'] + g['ml_bhh'])
    cl_wih, cl_whh = scale_g(g['cl_wih']), scale_g(g['cl_whh'])
    ml_wih, ml_whh = scale_g(g['ml_wih']), scale_g(g['ml_whh'])

    cd_wA, cd_wB = g['cd_w'][:, :H], g['cd_w'][:, H:]
    F = cd_wA @ g['ce_w']
    FW2 = F @ g['cp_w2']                       # [100,100]
    kc_bias = cd_wA @ g['ce_b'] + g['cd_b'] + F @ g['cp_b2']

    md_wA, md_wB, md_wC = g['md_w'][:, :H], g['md_w'][:, H:2 * H], g['md_w'][:, 2 * H:]
    P = md_wA @ g['me_w']                      # [100,130]
    PW2 = P @ g['mp_w2']                       # [100,100]
    km_bias = md_wA @ g['me_b'] + g['md_b'] + P @ g['mp_b2']

    def zcol(m, rows):
        """append a zero column (keeps row 100 of the psum at 0)."""
        return np.hstack([m, np.zeros((rows, 1))])

    def ecol(m, rows, one_at):
        c = np.zeros((rows, 1))
        c[one_at, 0] = 1.0
        return np.hstack([m, c])

    # kz stationaries get an e100 column: p[100] = ze[100] = 1
    kzc = ecol(np.vstack([cd_wB.T, kc_bias[None]]), 101, 100)       # [101,101]
    kzm = ecol(np.vstack([md_wC.T, km_bias[None]]), 101, 100)       # [101,101]

    p = {
        'w_cwx_c': np.vstack([cl_wih.T, b_c[None]]),        # [101,400]
        'w_cwh_c': cl_whh.T,                                # [100,400]
        'w_cp1': np.vstack([g['cp_w1'].T, g['cp_b1'][None]]),  # [101,100]
        'w_cp2': np.vstack([g['cp_w2'].T, g['cp_b2'][None]]),  # [101,25]
        'w_fw2': FW2.T,                                     # [100,100]
        'w_pw2': PW2.T,                                     # [100,100]
        'w_kzc': kzc, 'w_kzm': kzm,
        'w_mwx': np.vstack([ml_wih.T, b_m[None]]),          # [101,400]
        'w_mwh': ml_whh.T,                                  # [100,400]
        'w_mp1': np.vstack([g['mp_w1'].T, g['mp_b1'][None]]),  # [101,100]
        'w_mp2': np.vstack([g['mp_w2'].T, g['mp_b2'][None]]),  # [101,130]
        'w_mdb': zcol(md_wB.T, 100),                        # [100,101]
        'w_hd2k': g['key_w2'].T,                            # [50,24]
        'w_hd2m': g['mode_w2'].T,                           # [50,7]
        'w_hd2t': np.hstack([g['tp_w2'].T, g['vl_w2'].T, g['en_w2'].T]),  # [50,3]
        'w_hdbk': g['key_b2'][:, None],                     # [24,1]
        'w_hdbm': g['mode_b2'][:, None],                    # [7,1]
        'w_hdbt': np.concatenate([g['tp_b2'], g['vl_b2'], g['en_b2']])[None],  # [1,3]
        'z': g['z'],
    }
    hd1 = np.zeros((101, 250))
    for i, hd in enumerate(['key', 'mode', 'tp', 'vl', 'en']):
        hd1[:, 50 * i:50 * i + 50] = np.vstack(
            [g[f'{hd}_w1'].T, g[f'{hd}_b1'][None]])
    p['w_hd1'] = hd1
    return {k: np.ascontiguousarray(v, mm_dtype if k in LOOP_W else FP)
            for k, v in p.items()}


# ---------------------------------------------------------------- builder ---

def build(n_outer=N_OUTER, n_inner=N_INNER, n_cores=N_CORES,
          mm_dtype='float16'):
    """Build the Bass program. Returns nc."""
    from contextlib import ExitStack
    import concourse.bacc as bacc
    import concourse.mybir as mybir
    import concourse.tile as tile

    fp32 = mybir.dt.float32
    fpm = getattr(mybir.dt, mm_dtype)
    AF = mybir.ActivationFunctionType
    OP = mybir.AluOpType
    n_mel = n_outer * n_inner

    nc = bacc.Bacc("TRN2", target_bir_lowering=False, debug=False,
                   num_devices=n_cores)

    shapes = {
        'w_cwx_c': (101, 400), 'w_cwh_c': (100, 400), 'w_cp1': (101, 100),
        'w_cp2': (101, 25), 'w_fw2': (100, 100), 'w_kzc': (101, 101),
        'w_kzm': (101, 101), 'w_mwx': (101, 400), 'w_mwh': (100, 400),
        'w_mp1': (101, 100), 'w_mp2': (101, 130), 'w_pw2': (100, 100),
        'w_mdb': (100, 101), 'w_hd1': (101, 250),
        'w_hd2k': (50, 24), 'w_hd2m': (50, 7), 'w_hd2t': (50, 3),
        'w_hdbk': (24, 1), 'w_hdbm': (7, 1), 'w_hdbt': (1, 3),
        'z': (1, 100),
    }
    din = {k: nc.dram_tensor(k, s, fpm if k in LOOP_W else fp32,
                             kind="ExternalInput").ap()
           for k, s in shapes.items()}
    d_chord = nc.dram_tensor("chord_out", (n_outer, CP), fp32,
                             kind="ExternalOutput").ap()
    d_mel = nc.dram_tensor("mel_out", (n_mel, MP), fp32,
                           kind="ExternalOutput").ap()
    d_key = nc.dram_tensor("key_out", (1, NK), fp32, kind="ExternalOutput").ap()
    d_mode = nc.dram_tensor("mode_out", (1, NM), fp32, kind="ExternalOutput").ap()
    d_bpm = nc.dram_tensor("bpm", (1, 1), fp32, kind="ExternalOutput").ap()
    d_en = nc.dram_tensor("energy", (1, 1), fp32, kind="ExternalOutput").ap()
    d_vl = nc.dram_tensor("valence", (1, 1), fp32, kind="ExternalOutput").ap()

    with ExitStack() as ctx:
        tc = ctx.enter_context(tile.TileContext(nc))
        wp = ctx.enter_context(tc.tile_pool(name="w", bufs=1))
        sp = ctx.enter_context(tc.tile_pool(name="state", bufs=1))
        pp = ctx.enter_context(tc.tile_pool(name="ps", bufs=1, space="PSUM"))

        # --- weight tiles (DMA once) ---
        # Tiles read by steady-state matmuls are staged through a one-time
        # DVE copy: a matmul's waits then merge into the DVE sem instead of
        # keeping a forwarded DMA-queue wait alive forever (LDWEIGHTS has
        # only 2 HW sync-wait slots).
        wt = {}
        for k in shapes:
            if k == 'z':
                continue
            K, M = shapes[k]
            dt_k = fpm if k in LOOP_W else fp32
            stg = wp.tile([K, M], dt_k, name=f"stg_{k}")
            nc.sync.dma_start(out=stg[0:K, 0:M], in_=din[k])
            t = wp.tile([K, M], dt_k, name=f"t_{k}")
            nc.vector.tensor_copy(t[0:K, 0:M], stg[0:K, 0:M])
            wt[k] = t

        # --- state tiles ---
        ze = sp.tile([101, 1], fp32, name="ze")
        ze16 = sp.tile([101, 1], fpm, name="ze16")
        che = sp.tile([101, 1], fpm, name="che")
        hce = sp.tile([101, 1], fpm, name="hce")
        hme = sp.tile([101, 1], fpm, name="hme")
        xme = sp.tile([101, 1], fpm, name="xme")
        rce = sp.tile([101, 1], fpm, name="rce")
        rme = sp.tile([101, 1], fpm, name="rme")
        c_c = sp.tile([100, 1], fp32, name="c_c")
        c_m = sp.tile([100, 1], fp32, name="c_m")
        kz_c = sp.tile([101, 1], fp32, name="kz_c")
        kz_m = sp.tile([101, 1], fp32, name="kz_m")
        cvec = [sp.tile([101, 1], fp32, name=f"cvec{b}") for b in range(2)]
        e100 = sp.tile([101, 1], fp32, name="e100")
        seed = sp.tile([1, 1], fp32, name="seed")
        zrow = sp.tile([1, 101], fp32, name="zrow")
        erow = sp.tile([1, 101], fp32, name="erow")
        chord_o32 = sp.tile([25, n_outer], fp32, name="chord_o32")
        mel_o32 = sp.tile([65, 2 * n_mel], fp32, name="mel_o32")
        rh = sp.tile([50, 5], fp32, name="rh")
        key_sb = sp.tile([24, 1], fp32, name="key_sb")
        mode_sb = sp.tile([7, 1], fp32, name="mode_sb")
        bpm_sb = sp.tile([1, 3], fp32, name="bpm_sb")
        # persistent LSTM elementwise temps (one set per chain)
        sg_c = sp.tile([100, 4], fp32, name="sg_c")
        sg_m = sp.tile([100, 4], fp32, name="sg_m")
        tmp_c = [sp.tile([100, 1], fp32, name=f"tmp_c{i}") for i in range(3)]
        tmp_m = [sp.tile([100, 1], fp32, name=f"tmp_m{i}") for i in range(3)]

        # persistent PSUM tiles (8 banks): allocating per-step from a pool
        # would attach slot-release waits ({PE writers, ACT readers}) to the
        # first matmul of every step, busting the 2-slot sync-wait limit.
        gm = [pp.tile([100, 4], fp32, tag=f"gm{i}", name=f"gm{i}")
              for i in range(2)]
        pgc = pp.tile([100, 4], fp32, tag="pgc", name="pgc")
        p_c1 = pp.tile([101, 1], fp32, tag="pc1", name="p_c1")
        p_m1 = pp.tile([101, 1], fp32, tag="pm1", name="p_m1")
        p_m2 = pp.tile([65, 2], fp32, tag="pm2", name="p_m2")
        p_s = pp.tile([25, 3], fp32, tag="psm", name="p_s")
        p_hd = pp.tile([50, 5], fp32, tag="phd", name="p_hd")

        # --- bootstrap [z;1] and e100 columns (steady-state compute must not
        # read DMA-written data, so build them via K=1 matmuls) ---
        z_stg = sp.tile([1, 100], fp32, name="z_stg")
        nc.sync.dma_start(out=z_stg, in_=din['z'])
        nc.vector.tensor_copy(zrow[0:1, 0:100], z_stg)
        nc.vector.memset(zrow[0:1, 100:101], 1.0)
        nc.vector.memset(seed, 1.0)
        nc.vector.memset(erow[0:1, 0:100], 0.0)
        nc.vector.memset(erow[0:1, 100:101], 1.0)
        nc.tensor.matmul(p_c1, zrow, seed)
        nc.vector.tensor_copy(ze, p_c1)
        nc.tensor.matmul(p_c1, erow, seed)
        nc.vector.tensor_copy(e100, p_c1)
        nc.vector.tensor_copy(ze16, ze)
        # h/r state init: zeros with a 1.0 in row 100 (rows 0-99 rewritten per
        # step, row 100 persists)
        for t_ in (che, hce, hme, xme, rce, rme):
            nc.vector.tensor_copy(t_, e100)
        nc.vector.memset(c_c, 0.0)
        nc.vector.memset(c_m, 0.0)

        # --- init columns (z-dependent; stationaries carry an e100 column so
        # row 100 of kz comes out as 1.0) ---
        nc.tensor.matmul(p_c1, wt['w_kzc'][0:101, 0:101], ze[0:101, 0:1])
        nc.vector.tensor_copy(kz_c, p_c1)
        nc.tensor.matmul(p_c1, wt['w_kzm'][0:101, 0:101], ze[0:101, 0:1])
        nc.vector.tensor_copy(kz_m, p_c1)

        # --- heads ---
        for i in range(5):
            nc.tensor.matmul(p_hd[0:50, i:i + 1],
                             wt['w_hd1'][0:101, 50 * i:50 * i + 50],
                             ze[0:101, 0:1])
        nc.vector.tensor_scalar(rh[0:50, 0:5], p_hd[0:50, 0:5], 0.0, None,
                                OP.max)
        nc.tensor.matmul(p_s[0:24, 0:1], wt['w_hd2k'][0:50, 0:24],
                         rh[0:50, 0:1])
        nc.vector.tensor_add(key_sb, p_s[0:24, 0:1], wt['w_hdbk'][0:24, 0:1])
        nc.tensor.matmul(p_s[0:7, 1:2], wt['w_hd2m'][0:50, 0:7], rh[0:50, 1:2])
        nc.vector.tensor_add(mode_sb, p_s[0:7, 1:2], wt['w_hdbm'][0:7, 0:1])
        for j in range(3):
            nc.tensor.matmul(p_s[0:1, j:j + 1], wt['w_hd2t'][0:50, j:j + 1],
                             rh[0:50, 2 + j:3 + j])
        # bpm_sb cols: 0=bpm, 1=valence, 2=energy (all clipped)
        nc.vector.tensor_add(bpm_sb, p_s[0:1, 0:3], wt['w_hdbt'][0:1, 0:3])
        nc.vector.tensor_scalar(bpm_sb, bpm_sb, 0.0, 1.0, OP.max, OP.min)
        nc.vector.tensor_scalar(bpm_sb[0:1, 0:1], bpm_sb[0:1, 0:1],
                                30.0, 70.0, OP.mult, OP.add)
        nc.sync.dma_start(out=d_key.rearrange("o k -> k o"), in_=key_sb)
        nc.sync.dma_start(out=d_mode.rearrange("o k -> k o"), in_=mode_sb)
        nc.sync.dma_start(out=d_bpm, in_=bpm_sb[0:1, 0:1])
        nc.sync.dma_start(out=d_vl, in_=bpm_sb[0:1, 1:2])
        nc.sync.dma_start(out=d_en, in_=bpm_sb[0:1, 2:3])

        def lstm_elem(pg, c, h_dst, sg, tmps):
            """gates psum [100,4] cols (i,f,g*2,o) -> update c, h_dst[0:100].
            c' = 2*(sig_i*sig_2g) + (c*sig_f - sig_i); h' = sig_o*tanh(c')."""
            t1, t2, tc_ = tmps
            nc.scalar.activation(sg, pg, AF.Sigmoid)
            nc.vector.tensor_scalar(t1, sg[:, 2:3], sg[:, 0:1], None, OP.mult)
            nc.vector.scalar_tensor_tensor(t2, c, sg[:, 1:2], sg[:, 0:1],
                                           OP.mult, OP.subtract)
            nc.vector.scalar_tensor_tensor(c, t1, 2.0, t2, OP.mult, OP.add)
            nc.scalar.activation(tc_, c, AF.Tanh)
            nc.vector.tensor_scalar(h_dst[0:100, 0:1], tc_, sg[:, 3:4],
                                    None, OP.mult)

        for t in range(n_outer):
            # ---- chord step t ----
            rhs_x = ze16 if t == 0 else che
            pg = pgc
            for j in range(4):
                nc.tensor.matmul(pg[:, j:j + 1],
                                 wt['w_cwx_c'][0:101, 100 * j:100 * j + 100],
                                 rhs_x[0:101, 0:1], start=True, stop=(t == 0))
                if t > 0:
                    nc.tensor.matmul(pg[:, j:j + 1],
                                     wt['w_cwh_c'][0:100, 100 * j:100 * j + 100],
                                     hce[0:100, 0:1], start=False, stop=True)
            lstm_elem(pg, c_c, hce, sg_c, tmp_c)
            nc.tensor.matmul(p_m1[0:100, 0:1], wt['w_cp1'][0:101, 0:100],
                             hce[0:101, 0:1])
            nc.vector.tensor_scalar(rce[0:100, 0:1], p_m1[0:100, 0:1], 0.0,
                                    None, OP.max)
            nc.tensor.matmul(p_s[0:25, 0:1], wt['w_cp2'][0:101, 0:25],
                             rce[0:101, 0:1])
            nc.scalar.copy(chord_o32[0:25, t:t + 1], p_s[0:25, 0:1])
            nc.tensor.matmul(p_c1[0:100, 0:1], wt['w_fw2'][0:100, 0:100],
                             rce[0:100, 0:1])
            nc.vector.scalar_tensor_tensor(che[0:100, 0:1], p_c1[0:100, 0:1],
                                           1.0, kz_c[0:100, 0:1],
                                           OP.mult, OP.add)

            # ---- cvec for outer t (row 100 = 0 + kz_m[100] = 1) ----
            cv = cvec[t % 2]
            nc.tensor.matmul(p_c1, wt['w_mdb'][0:100, 0:101], che[0:100, 0:1])
            nc.vector.scalar_tensor_tensor(cv, p_c1, 1.0, kz_m, OP.mult, OP.add)

            # ---- melody steps ----
            for s in range(n_inner * t, n_inner * (t + 1)):
                pgm = gm[s % 2]
                rhs_m = che if s == 0 else xme
                for j in range(4):
                    nc.tensor.matmul(
                        pgm[:, j:j + 1],
                        wt['w_mwx'][0:101, 100 * j:100 * j + 100],
                        rhs_m[0:101, 0:1], start=True, stop=(s == 0))
                    if s > 0:
                        nc.tensor.matmul(
                            pgm[:, j:j + 1],
                            wt['w_mwh'][0:100, 100 * j:100 * j + 100],
                            hme[0:100, 0:1], start=False, stop=True)
                lstm_elem(pgm, c_m, hme, sg_m, tmp_m)
                nc.tensor.matmul(p_m1[0:100, 0:1], wt['w_mp1'][0:101, 0:100],
                                 hme[0:101, 0:1])
                nc.vector.tensor_scalar(rme[0:100, 0:1], p_m1[0:100, 0:1],
                                        0.0, None, OP.max)
                # next x = PW2 @ relu + cvec (mel_pred folded through); the
                # mel output matmuls run off the critical path
                nc.tensor.matmul(p_m1[0:100, 0:1], wt['w_pw2'][0:100, 0:100],
                                 rme[0:100, 0:1])
                nc.vector.scalar_tensor_tensor(xme[0:100, 0:1],
                                               p_m1[0:100, 0:1], 1.0,
                                               cv[0:100, 0:1],
                                               OP.mult, OP.add)
                nc.tensor.matmul(p_m2[:, 0:1], wt['w_mp2'][0:101, 0:65],
                                 rme[0:101, 0:1])
                nc.tensor.matmul(p_m2[:, 1:2], wt['w_mp2'][0:101, 65:130],
                                 rme[0:101, 0:1])
                nc.scalar.copy(mel_o32[0:65, 2 * s:2 * s + 2], p_m2)

        # ---- outputs ----
        nc.sync.dma_start(out=d_chord.rearrange("t p -> p t"),
                          in_=chord_o32[0:25, 0:n_outer])
        nc.sync.dma_start(out=d_mel.rearrange("s (two p) -> p s two", two=2),
                          in_=mel_o32[0:65, 0:2 * n_mel].rearrange(
                              "p (s two) -> p s two", two=2))
    nc.compile()
    return nc


# ----------------------------------------------------------------- runner ---

_CACHE = {}


def _enable_ntff_tracing():
    """Register the axon NTFF profile hook if the container's antenv lacks it,
    and neuter the bucket upload. Only needed for trace=True runs."""
    import sys
    import types
    try:
        from antenv.axon_hooks import get_axon_ntff_profile_hook  # noqa: F401
        have = True
    except ImportError:
        have = False
    if not have:
        from trn_agent_boot.trn_boot import _ntff_profile_via_ctypes
        hook = _ntff_profile_via_ctypes('/opt/axon/libaxon_pjrt.so')
        mod = types.ModuleType('antenv.axon_hooks')
        mod.get_axon_ntff_profile_hook = lambda: hook
        mod.set_axon_ntff_profile_hook = lambda h: None
        sys.modules['antenv.axon_hooks'] = mod
    import concourse.bass_utils as bu
    bu.upload_artifacts = lambda tmpdir: f"local://{tmpdir}"


MM_DTYPE = 'float16'


def _get_nc(n_outer, n_inner, n_cores, mm_dtype=MM_DTYPE):
    k = (n_outer, n_inner, n_cores, mm_dtype)
    if k not in _CACHE:
        _CACHE[k] = build(n_outer, n_inner, n_cores, mm_dtype)
    return _CACHE[k]


def run(inputs, n_outer=N_OUTER, n_inner=N_INNER, n_cores=N_CORES,
        trace=False, mm_dtype=MM_DTYPE):
    """Run on hardware; returns (outputs_tuple, BassKernelResults)."""
    import numpy as _np
    from concourse.bass_utils import run_bass_kernel_spmd
    if trace:
        try:
            _enable_ntff_tracing()
        except Exception as e:  # degrade to no-trace
            print(f'ntff tracing unavailable: {e}')
            trace = False
    nc = _get_nc(n_outer, n_inner, n_cores, mm_dtype)
    in_map = pack_inputs(inputs, getattr(_np, mm_dtype))
    res = run_bass_kernel_spmd(nc, [dict(in_map) for _ in range(n_cores)],
                               core_ids=list(range(n_cores)), trace=trace)
    r = res.results[0]
    n_mel = n_outer * n_inner
    out = (r['chord_out'].reshape(n_outer, 1, CP).astype(FP),
           r['mel_out'].reshape(n_mel, 1, MP).astype(FP),
           r['key_out'].astype(FP), r['mode_out'].astype(FP),
           r['bpm'].astype(FP), r['energy'].astype(FP),
           r['valence'].astype(FP))
    return out, res


def kernel(**inputs):
    out, _ = run(inputs)
    return out


# revision 24
# speedup vs baseline: 2.4089x; 1.0661x over previous
"""Trainium2 Bass kernel for nn_Decoder2 (sequential LSTM music decoder).

Strategy (per sharding hint): single-core persistent kernel — all weights
resident in SBUF, state kept as column vectors across partitions, fully
unrolled 50-step chord chain + 400-step melody chain with host-side weight
folding. Inputs are replicated across the 8 cores (SPMD, identical program);
core 0's output is returned.

Host-side folds (weights only; z-dependent vectors are computed on device):
  - chord:  ch_emb(t) = F @ chord_pred(t) + kz_c,  F = cd_wA @ ce_w,
            kz_c = cd_wB @ z + cd_wA @ ce_b + cd_b  (on device, once).
  - melody: x(s+1) = P @ mel_pred(s) + cvec(t),  P = md_wA @ me_w,
            cvec(t) = md_wB @ ch_emb(t) + kz_m,
            kz_m = md_wC @ z + md_wA @ me_b + md_b  (on device, once).
  - biases folded into row-100 of the stationaries; state rhs vectors carry a
    constant 1.0 in row 100. Compute engines need 32-aligned partition bases,
    so row 100 is produced arithmetically: bootstrap [z;1] / e100 columns via
    K=1 matmuls from a [1,M] row, and keep the 1 alive through the kz/cvec
    chain (kz vectors have a 1 in row 100 via an e100 column in their
    stationaries; intermediate psums have a zero column there).
  - matmuls (LDWEIGHTS) have only 2 HW sync-wait slots, so every loop matmul
    must depend on at most {one DMA-written weight tile, DVE-written data}.
  - g-gate blocks pre-scaled by 2 so one sigmoid serves all 4 gates
    (tanh(x) = 2*sigmoid(2x) - 1, exact identity).
"""
import numpy as np

H, H2, CP, MP, NK, NM = 100, 50, 25, 130, 24, 7
N_OUTER, N_INNER = 50, 8
N_CORES = 8

FP = np.float32


# ---------------------------------------------------------------- packing ---

LOOP_W = {'w_cwx_c', 'w_cwh_c', 'w_cp1', 'w_cp2', 'w_fw2', 'w_pw2',
          'w_mwx', 'w_mwh', 'w_mp1', 'w_mp2', 'w_mdb'}


def pack_inputs(inp, mm_dtype=np.float16):
    """Fold/transpose raw weights into the DRAM layout the kernel expects.
    Loop-matmul weights are cast to mm_dtype (fp32 matmuls run at 1/4 rate on
    the PE); init/head weights stay fp32."""
    g = {k: np.asarray(v, np.float64) for k, v in inp.items()}

    def scale_g(w400):
        w = w400.copy()
        w[200:300] *= 2.0
        return w

    b_c = scale_g(g['cl_bih'] + g['cl_bhh'])
    b_m = scale_g(g['ml_bih'] + g['ml_bhh'])
    cl_wih, cl_whh = scale_g(g['cl_wih']), scale_g(g['cl_whh'])
    ml_wih, ml_whh = scale_g(g['ml_wih']), scale_g(g['ml_whh'])

    cd_wA, cd_wB = g['cd_w'][:, :H], g['cd_w'][:, H:]
    F = cd_wA @ g['ce_w']
    FW2 = F @ g['cp_w2']                       # [100,100]
    kc_bias = cd_wA @ g['ce_b'] + g['cd_b'] + F @ g['cp_b2']

    md_wA, md_wB, md_wC = g['md_w'][:, :H], g['md_w'][:, H:2 * H], g['md_w'][:, 2 * H:]
    P = md_wA @ g['me_w']                      # [100,130]
    PW2 = P @ g['mp_w2']                       # [100,100]
    km_bias = md_wA @ g['me_b'] + g['md_b'] + P @ g['mp_b2']

    def zcol(m, rows):
        """append a zero column (keeps row 100 of the psum at 0)."""
        return np.hstack([m, np.zeros((rows, 1))])

    def ecol(m, rows, one_at):
        c = np.zeros((rows, 1))
        c[one_at, 0] = 1.0
        return np.hstack([m, c])

    # kz stationaries get an e100 column: p[100] = ze[100] = 1
    kzc = ecol(np.vstack([cd_wB.T, kc_bias[None]]), 101, 100)       # [101,101]
    kzm = ecol(np.vstack([md_wC.T, km_bias[None]]), 101, 100)       # [101,101]

    p = {
        'w_cwx_c': np.vstack([cl_wih.T, b_c[None]]),        # [101,400]
        'w_cwh_c': cl_whh.T,                                # [100,400]
        'w_cp1': np.vstack([g['cp_w1'].T, g['cp_b1'][None]]),  # [101,100]
        'w_cp2': np.vstack([g['cp_w2'].T, g['cp_b2'][None]]),  # [101,25]
        'w_fw2': FW2.T,                                     # [100,100]
        'w_pw2': PW2.T,                                     # [100,100]
        'w_kzc': kzc, 'w_kzm': kzm,
        'w_mwx': np.vstack([ml_wih.T, b_m[None]]),          # [101,400]
        'w_mwh': ml_whh.T,                                  # [100,400]
        'w_mp1': np.vstack([g['mp_w1'].T, g['mp_b1'][None]]),  # [101,100]
        'w_mp2': np.vstack([g['mp_w2'].T, g['mp_b2'][None]]),  # [101,130]
        'w_mdb': zcol(md_wB.T, 100),                        # [100,101]
        'w_hd2k': g['key_w2'].T,                            # [50,24]
        'w_hd2m': g['mode_w2'].T,                           # [50,7]
        'w_hd2t': np.hstack([g['tp_w2'].T, g['vl_w2'].T, g['en_w2'].T]),  # [50,3]
        'w_hdbk': g['key_b2'][:, None],                     # [24,1]
        'w_hdbm': g['mode_b2'][:, None],                    # [7,1]
        'w_hdbt': np.concatenate([g['tp_b2'], g['vl_b2'], g['en_b2']])[None],  # [1,3]
        'z': g['z'],
    }
    hd1 = np.zeros((101, 250))
    for i, hd in enumerate(['key', 'mode', 'tp', 'vl', 'en']):
        hd1[:, 50 * i:50 * i + 50] = np.vstack(
            [g[f'{hd}_w1'].T, g[f'{hd}_b1'][None]])
    p['w_hd1'] = hd1
    return {k: np.ascontiguousarray(v, mm_dtype if k in LOOP_W else FP)
            for k, v in p.items()}


# ---------------------------------------------------------------- builder ---

def build(n_outer=N_OUTER, n_inner=N_INNER, n_cores=N_CORES,
          mm_dtype='float16'):
    """Build the Bass program. Returns nc."""
    from contextlib import ExitStack
    import concourse.bacc as bacc
    import concourse.mybir as mybir
    import concourse.tile as tile

    fp32 = mybir.dt.float32
    fpm = getattr(mybir.dt, mm_dtype)
    AF = mybir.ActivationFunctionType
    OP = mybir.AluOpType
    n_mel = n_outer * n_inner

    nc = bacc.Bacc("TRN2", target_bir_lowering=False, debug=False,
                   num_devices=n_cores)

    shapes = {
        'w_cwx_c': (101, 400), 'w_cwh_c': (100, 400), 'w_cp1': (101, 100),
        'w_cp2': (101, 25), 'w_fw2': (100, 100), 'w_kzc': (101, 101),
        'w_kzm': (101, 101), 'w_mwx': (101, 400), 'w_mwh': (100, 400),
        'w_mp1': (101, 100), 'w_mp2': (101, 130), 'w_pw2': (100, 100),
        'w_mdb': (100, 101), 'w_hd1': (101, 250),
        'w_hd2k': (50, 24), 'w_hd2m': (50, 7), 'w_hd2t': (50, 3),
        'w_hdbk': (24, 1), 'w_hdbm': (7, 1), 'w_hdbt': (1, 3),
        'z': (1, 100),
    }
    din = {k: nc.dram_tensor(k, s, fpm if k in LOOP_W else fp32,
                             kind="ExternalInput").ap()
           for k, s in shapes.items()}
    d_chord = nc.dram_tensor("chord_out", (n_outer, CP), fp32,
                             kind="ExternalOutput").ap()
    d_mel = nc.dram_tensor("mel_out", (n_mel, MP), fp32,
                           kind="ExternalOutput").ap()
    d_key = nc.dram_tensor("key_out", (1, NK), fp32, kind="ExternalOutput").ap()
    d_mode = nc.dram_tensor("mode_out", (1, NM), fp32, kind="ExternalOutput").ap()
    d_bpm = nc.dram_tensor("bpm", (1, 1), fp32, kind="ExternalOutput").ap()
    d_en = nc.dram_tensor("energy", (1, 1), fp32, kind="ExternalOutput").ap()
    d_vl = nc.dram_tensor("valence", (1, 1), fp32, kind="ExternalOutput").ap()

    with ExitStack() as ctx:
        tc = ctx.enter_context(tile.TileContext(nc))
        wp = ctx.enter_context(tc.tile_pool(name="w", bufs=1))
        sp = ctx.enter_context(tc.tile_pool(name="state", bufs=1))
        pp = ctx.enter_context(tc.tile_pool(name="ps", bufs=1, space="PSUM"))

        # --- weight tiles (DMA once) ---
        # Tiles read by steady-state matmuls are staged through a one-time
        # DVE copy: a matmul's waits then merge into the DVE sem instead of
        # keeping a forwarded DMA-queue wait alive forever (LDWEIGHTS has
        # only 2 HW sync-wait slots).
        wt = {}
        for k in shapes:
            if k == 'z':
                continue
            K, M = shapes[k]
            dt_k = fpm if k in LOOP_W else fp32
            stg = wp.tile([K, M], dt_k, name=f"stg_{k}")
            nc.sync.dma_start(out=stg[0:K, 0:M], in_=din[k])
            t = wp.tile([K, M], dt_k, name=f"t_{k}")
            nc.vector.tensor_copy(t[0:K, 0:M], stg[0:K, 0:M])
            wt[k] = t

        # --- state tiles ---
        ze = sp.tile([101, 1], fp32, name="ze")
        ze16 = sp.tile([101, 1], fpm, name="ze16")
        che = sp.tile([101, 1], fpm, name="che")
        hce = sp.tile([101, 1], fpm, name="hce")
        hme = sp.tile([101, 1], fpm, name="hme")
        xme = sp.tile([101, 1], fpm, name="xme")
        rce = sp.tile([101, 1], fpm, name="rce")
        rme = sp.tile([101, 1], fpm, name="rme")
        c_c = sp.tile([100, 1], fp32, name="c_c")
        c_m = sp.tile([100, 1], fp32, name="c_m")
        kz_c = sp.tile([101, 1], fp32, name="kz_c")
        kz_m = sp.tile([101, 1], fp32, name="kz_m")
        cvec = [sp.tile([101, 1], fp32, name=f"cvec{b}") for b in range(2)]
        e100 = sp.tile([101, 1], fp32, name="e100")
        seed = sp.tile([1, 1], fp32, name="seed")
        zrow = sp.tile([1, 101], fp32, name="zrow")
        erow = sp.tile([1, 101], fp32, name="erow")
        chord_o32 = sp.tile([25, n_outer], fp32, name="chord_o32")
        mel_o32 = sp.tile([65, 2 * n_mel], fp32, name="mel_o32")
        rh = sp.tile([50, 5], fp32, name="rh")
        key_sb = sp.tile([24, 1], fp32, name="key_sb")
        mode_sb = sp.tile([7, 1], fp32, name="mode_sb")
        bpm_sb = sp.tile([1, 3], fp32, name="bpm_sb")
        # persistent LSTM elementwise temps (one set per chain)
        sg_c = sp.tile([100, 4], fp32, name="sg_c")
        sg_m = sp.tile([100, 4], fp32, name="sg_m")
        tmp_c = [sp.tile([100, 1], fp32, name=f"tmp_c{i}") for i in range(3)]
        tmp_m = [sp.tile([100, 1], fp32, name=f"tmp_m{i}") for i in range(3)]

        # persistent PSUM tiles (8 banks): allocating per-step from a pool
        # would attach slot-release waits ({PE writers, ACT readers}) to the
        # first matmul of every step, busting the 2-slot sync-wait limit.
        gm = [pp.tile([100, 4], fp32, tag=f"gm{i}", name=f"gm{i}")
              for i in range(2)]
        pgc = pp.tile([100, 4], fp32, tag="pgc", name="pgc")
        p_c1 = pp.tile([101, 1], fp32, tag="pc1", name="p_c1")
        p_m1 = pp.tile([101, 1], fp32, tag="pm1", name="p_m1")
        p_m2 = pp.tile([65, 2], fp32, tag="pm2", name="p_m2")
        p_s = pp.tile([25, 3], fp32, tag="psm", name="p_s")
        p_hd = pp.tile([50, 5], fp32, tag="phd", name="p_hd")

        # --- bootstrap [z;1] and e100 columns (steady-state compute must not
        # read DMA-written data, so build them via K=1 matmuls) ---
        z_stg = sp.tile([1, 100], fp32, name="z_stg")
        nc.sync.dma_start(out=z_stg, in_=din['z'])
        nc.vector.tensor_copy(zrow[0:1, 0:100], z_stg)
        nc.vector.memset(zrow[0:1, 100:101], 1.0)
        nc.vector.memset(seed, 1.0)
        nc.vector.memset(erow[0:1, 0:100], 0.0)
        nc.vector.memset(erow[0:1, 100:101], 1.0)
        nc.tensor.matmul(p_c1, zrow, seed)
        nc.vector.tensor_copy(ze, p_c1)
        nc.tensor.matmul(p_c1, erow, seed)
        nc.vector.tensor_copy(e100, p_c1)
        nc.vector.tensor_copy(ze16, ze)
        # h/r state init: zeros with a 1.0 in row 100 (rows 0-99 rewritten per
        # step, row 100 persists)
        for t_ in (che, hce, hme, xme, rce, rme):
            nc.vector.tensor_copy(t_, e100)
        nc.vector.memset(c_c, 0.0)
        nc.vector.memset(c_m, 0.0)

        # --- init columns (z-dependent; stationaries carry an e100 column so
        # row 100 of kz comes out as 1.0) ---
        nc.tensor.matmul(p_c1, wt['w_kzc'][0:101, 0:101], ze[0:101, 0:1])
        nc.vector.tensor_copy(kz_c, p_c1)
        nc.tensor.matmul(p_c1, wt['w_kzm'][0:101, 0:101], ze[0:101, 0:1])
        nc.vector.tensor_copy(kz_m, p_c1)

        # --- heads ---
        for i in range(5):
            nc.tensor.matmul(p_hd[0:50, i:i + 1],
                             wt['w_hd1'][0:101, 50 * i:50 * i + 50],
                             ze[0:101, 0:1])
        nc.vector.tensor_scalar(rh[0:50, 0:5], p_hd[0:50, 0:5], 0.0, None,
                                OP.max)
        nc.tensor.matmul(p_s[0:24, 0:1], wt['w_hd2k'][0:50, 0:24],
                         rh[0:50, 0:1])
        nc.vector.tensor_add(key_sb, p_s[0:24, 0:1], wt['w_hdbk'][0:24, 0:1])
        nc.tensor.matmul(p_s[0:7, 1:2], wt['w_hd2m'][0:50, 0:7], rh[0:50, 1:2])
        nc.vector.tensor_add(mode_sb, p_s[0:7, 1:2], wt['w_hdbm'][0:7, 0:1])
        for j in range(3):
            nc.tensor.matmul(p_s[0:1, j:j + 1], wt['w_hd2t'][0:50, j:j + 1],
                             rh[0:50, 2 + j:3 + j])
        # bpm_sb cols: 0=bpm, 1=valence, 2=energy (all clipped)
        nc.vector.tensor_add(bpm_sb, p_s[0:1, 0:3], wt['w_hdbt'][0:1, 0:3])
        nc.vector.tensor_scalar(bpm_sb, bpm_sb, 0.0, 1.0, OP.max, OP.min)
        nc.vector.tensor_scalar(bpm_sb[0:1, 0:1], bpm_sb[0:1, 0:1],
                                30.0, 70.0, OP.mult, OP.add)
        nc.sync.dma_start(out=d_key.rearrange("o k -> k o"), in_=key_sb)
        nc.sync.dma_start(out=d_mode.rearrange("o k -> k o"), in_=mode_sb)
        nc.sync.dma_start(out=d_bpm, in_=bpm_sb[0:1, 0:1])
        nc.sync.dma_start(out=d_vl, in_=bpm_sb[0:1, 1:2])
        nc.sync.dma_start(out=d_en, in_=bpm_sb[0:1, 2:3])

        def lstm_elem(pg, c, h_dst, sg, tmps):
            """gates psum [100,4] cols (i,f,g*2,o) -> update c, h_dst[0:100].
            c' = 2*(sig_i*sig_2g) + (c*sig_f - sig_i); h' = sig_o*tanh(c')."""
            t1, t2, tc_ = tmps
            nc.scalar.activation(sg, pg, AF.Sigmoid)
            nc.vector.scalar_tensor_tensor(t1, sg[:, 2:3], 2.0, sg[:, 0:1],
                                           OP.mult, OP.mult)
            nc.vector.scalar_tensor_tensor(t2, c, sg[:, 1:2], sg[:, 0:1],
                                           OP.mult, OP.subtract)
            nc.vector.scalar_tensor_tensor(c, t1, 1.0, t2, OP.mult, OP.add)
            nc.scalar.activation(tc_, c, AF.Tanh)
            nc.vector.scalar_tensor_tensor(h_dst[0:100, 0:1], tc_, 1.0,
                                           sg[:, 3:4], OP.mult, OP.mult)

        for t in range(n_outer):
            # ---- chord step t ----
            rhs_x = ze16 if t == 0 else che
            pg = pgc
            for j in range(4):
                if t > 0:
                    nc.tensor.matmul(pg[:, j:j + 1],
                                     wt['w_cwh_c'][0:100, 100 * j:100 * j + 100],
                                     hce[0:100, 0:1], start=True, stop=False)
                nc.tensor.matmul(pg[:, j:j + 1],
                                 wt['w_cwx_c'][0:101, 100 * j:100 * j + 100],
                                 rhs_x[0:101, 0:1], start=(t == 0), stop=True)
            lstm_elem(pg, c_c, hce, sg_c, tmp_c)
            nc.tensor.matmul(p_c1[0:100, 0:1], wt['w_cp1'][0:101, 0:100],
                             hce[0:101, 0:1])
            nc.vector.tensor_scalar(rce[0:100, 0:1], p_c1[0:100, 0:1], 0.0,
                                    None, OP.max)
            nc.tensor.matmul(p_s[0:25, 0:1], wt['w_cp2'][0:101, 0:25],
                             rce[0:101, 0:1])
            nc.vector.tensor_copy(chord_o32[0:25, t:t + 1], p_s[0:25, 0:1])
            nc.tensor.matmul(p_c1[0:100, 0:1], wt['w_fw2'][0:100, 0:100],
                             rce[0:100, 0:1])
            nc.vector.scalar_tensor_tensor(che[0:100, 0:1], p_c1[0:100, 0:1],
                                           1.0, kz_c[0:100, 0:1],
                                           OP.mult, OP.add)

            # ---- cvec for outer t (row 100 = 0 + kz_m[100] = 1) ----
            cv = cvec[t % 2]
            nc.tensor.matmul(p_c1, wt['w_mdb'][0:100, 0:101], che[0:100, 0:1])
            nc.vector.scalar_tensor_tensor(cv, p_c1, 1.0, kz_m, OP.mult, OP.add)

            # ---- melody steps ----
            for s in range(n_inner * t, n_inner * (t + 1)):
                pgm = gm[s % 2]
                rhs_m = che if s == 0 else xme
                for j in range(4):
                    # h-side first: h is ready before x, so these hide in the
                    # previous step's tail
                    if s > 0:
                        nc.tensor.matmul(
                            pgm[:, j:j + 1],
                            wt['w_mwh'][0:100, 100 * j:100 * j + 100],
                            hme[0:100, 0:1], start=True, stop=False)
                    nc.tensor.matmul(
                        pgm[:, j:j + 1],
                        wt['w_mwx'][0:101, 100 * j:100 * j + 100],
                        rhs_m[0:101, 0:1], start=(s == 0), stop=True)
                lstm_elem(pgm, c_m, hme, sg_m, tmp_m)
                nc.tensor.matmul(p_m1[0:100, 0:1], wt['w_mp1'][0:101, 0:100],
                                 hme[0:101, 0:1])
                nc.vector.tensor_scalar(rme[0:100, 0:1], p_m1[0:100, 0:1],
                                        0.0, None, OP.max)
                # next x = PW2 @ relu + cvec (mel_pred folded through); the
                # mel output matmuls run off the critical path
                nc.tensor.matmul(p_m1[0:100, 0:1], wt['w_pw2'][0:100, 0:100],
                                 rme[0:100, 0:1])
                nc.vector.scalar_tensor_tensor(xme[0:100, 0:1],
                                               p_m1[0:100, 0:1], 1.0,
                                               cv[0:100, 0:1],
                                               OP.mult, OP.add)
                nc.tensor.matmul(p_m2[:, 0:1], wt['w_mp2'][0:101, 0:65],
                                 rme[0:101, 0:1])
                nc.tensor.matmul(p_m2[:, 1:2], wt['w_mp2'][0:101, 65:130],
                                 rme[0:101, 0:1])
                nc.vector.tensor_copy(mel_o32[0:65, 2 * s:2 * s + 2], p_m2)

        # ---- outputs ----
        nc.sync.dma_start(out=d_chord.rearrange("t p -> p t"),
                          in_=chord_o32[0:25, 0:n_outer])
        nc.sync.dma_start(out=d_mel.rearrange("s (two p) -> p s two", two=2),
                          in_=mel_o32[0:65, 0:2 * n_mel].rearrange(
                              "p (s two) -> p s two", two=2))
    nc.compile()
    return nc


# ----------------------------------------------------------------- runner ---

_CACHE = {}


def _enable_ntff_tracing():
    """Register the axon NTFF profile hook if the container's antenv lacks it,
    and neuter the bucket upload. Only needed for trace=True runs."""
    import sys
    import types
    try:
        from antenv.axon_hooks import get_axon_ntff_profile_hook  # noqa: F401
        have = True
    except ImportError:
        have = False
    if not have:
        from trn_agent_boot.trn_boot import _ntff_profile_via_ctypes
        hook = _ntff_profile_via_ctypes('/opt/axon/libaxon_pjrt.so')
        mod = types.ModuleType('antenv.axon_hooks')
        mod.get_axon_ntff_profile_hook = lambda: hook
        mod.set_axon_ntff_profile_hook = lambda h: None
        sys.modules['antenv.axon_hooks'] = mod
    import concourse.bass_utils as bu
    bu.upload_artifacts = lambda tmpdir: f"local://{tmpdir}"


MM_DTYPE = 'float16'


def _get_nc(n_outer, n_inner, n_cores, mm_dtype=MM_DTYPE):
    k = (n_outer, n_inner, n_cores, mm_dtype)
    if k not in _CACHE:
        _CACHE[k] = build(n_outer, n_inner, n_cores, mm_dtype)
    return _CACHE[k]


def run(inputs, n_outer=N_OUTER, n_inner=N_INNER, n_cores=N_CORES,
        trace=False, mm_dtype=MM_DTYPE):
    """Run on hardware; returns (outputs_tuple, BassKernelResults)."""
    import numpy as _np
    from concourse.bass_utils import run_bass_kernel_spmd
    if trace:
        try:
            _enable_ntff_tracing()
        except Exception as e:  # degrade to no-trace
            print(f'ntff tracing unavailable: {e}')
            trace = False
    nc = _get_nc(n_outer, n_inner, n_cores, mm_dtype)
    in_map = pack_inputs(inputs, getattr(_np, mm_dtype))
    res = run_bass_kernel_spmd(nc, [dict(in_map) for _ in range(n_cores)],
                               core_ids=list(range(n_cores)), trace=trace)
    r = res.results[0]
    n_mel = n_outer * n_inner
    out = (r['chord_out'].reshape(n_outer, 1, CP).astype(FP),
           r['mel_out'].reshape(n_mel, 1, MP).astype(FP),
           r['key_out'].astype(FP), r['mode_out'].astype(FP),
           r['bpm'].astype(FP), r['energy'].astype(FP),
           r['valence'].astype(FP))
    return out, res


def kernel(**inputs):
    out, _ = run(inputs)
    return out
